# revision 20
# baseline (speedup 1.0000x reference)
"""Trainium2 Bass kernel for a locally-connected Conv2d (nn.Conv2dLocal).

Problem shapes (hardcoded):
  x      [B=64, Cin=32, H=32, W=32]  fp32
  weight [OH=30, OW=30, Cout=64, Cin=32, KH=3, KW=3] fp32 (per-location weights)
  bias   [Cout=64, OH=30, OW=30] fp32
  out    [B=64, Cout=64, OH=30, OW=30] fp32

Strategy: shard the 30 output rows across 8 cores (4 padded rows per core);
all device compute runs in bf16 (inputs quantized on host; rel-err budget is
2e-2, bf16 end-to-end measures ~3.5e-3).

Per core the whole working set fits in SBUF at once, so there is no pool
rotation and every input transfer is dependency-free and issued up front.
Per output row h, an SBUF tile xh[h] holds three input image rows as
[(kh, ci) -> 96 partitions + ones row, (img col c, batch b) free].  For each
image column c the stationary operand xh[:, c] is shared by up to three
(wl, kw) weight taps (wl + kw == c); the per-location weights stream as the
moving operand from one [97, 4*5760] tile (per-h panels side by side),
packed on the host so each step's taps are contiguous.  Accumulation is in
PSUM: one bank holds 8 output locations (64 cols each); per-element
has_written bits make the first tap overwrite and later taps accumulate.
The bias rides as a 97th contraction row against the ones row — matmul time
depends only on the moving free size, so the bias is free on the PE.

DMA rings: sync carries the four [96, 4KB] x mains; scalar carries eight
[96, 5760B] weight half-panels (the PE chases this stream); gpsimd carries
the tiny single-partition ones/bias rows (up front) and the dep-gated per-h
output ships.  96-row transfers fan across all 16 SDMA engines.
"""

import os
import sys

import numpy as np

for _p in ("/opt/trn_rl_repo", "/root/.axon_site/_ro/trn_rl_repo"):
    if os.path.isdir(_p) and _p not in sys.path:
        sys.path.insert(0, _p)

import concourse.bass as bass  # noqa: E402
import concourse.tile as tile  # noqa: E402
from concourse import bacc, mybir  # noqa: E402
from concourse.bass_utils import run_bass_kernel_spmd  # noqa: E402

import ml_dtypes  # noqa: E402

F32 = mybir.dt.float32
BF16 = mybir.dt.bfloat16
NP_BF16 = ml_dtypes.bfloat16

# problem constants
B, CI, H, W = 64, 32, 32, 32
CO = 64
KH = KW = 3
OH = OW = 30
NCORES = 8
RPC = 4  # padded output rows per core (8 * 4 = 32 >= 30)
OHP = NCORES * RPC  # 32
HPAD = OHP + KH - 1  # 34 padded input rows
K96 = KH * CI  # 96 contraction rows per kw tap
KP = K96 + 1  # + ones row for bias
XROWS = (RPC + 2) * CI  # 192 x rows per core (+1 ones row appended)

# (c, j, wl) pair enumeration: j descending within each c so that psum slots
# (wl % 8) ascend within a segment, matching the moving-operand column order.
PAIRS = []
for _c in range(W):
    for _j in (2, 1, 0):
        _wl = _c - _j
        if 0 <= _wl < OW:
            PAIRS.append((_c, _j, _wl))
NPAIRS = len(PAIRS)  # 90
WPANEL = NPAIRS * CO  # 5760 elements per h panel


def _build_segments():
    """Per-c matmul segments: lists of consecutive (pair_idx, (c, j, wl)).

    A segment's taps land in one PSUM bank with ascending slots.  start=True
    is used only for the matmul that is the first write of a whole bank
    (slot 0's j==0 tap) — it clears the bank's has_written bits; hardware's
    per-element has_written bits then make each slot's first tap overwrite
    and later taps accumulate.
    """
    segments = {c: [] for c in range(W)}
    for c in range(W):
        pairs = [(i, PAIRS[i]) for i in range(NPAIRS) if PAIRS[i][0] == c]
        seg = []
        for i, (cc, j, wl) in pairs:
            if seg and (seg[-1][1][2] // 8) != (wl // 8):
                segments[c].append(seg)
                seg = []
            seg.append((i, (cc, j, wl)))
        if seg:
            segments[c].append(seg)
    return segments


# drain bank `beta` right after processing column c == last write for the bank
DRAIN_AFTER_C = {}
for beta in range(4):
    last_wl = min(8 * beta + 7, OW - 1)
    DRAIN_AFTER_C.setdefault(last_wl + 2, []).append(beta)

_CACHED = {}


def _build_nc():
    """Build the single-core SPMD Bass program (identical on all 8 cores)."""
    from contextlib import ExitStack

    segments = _build_segments()
    nc = bacc.Bacc("TRN2", target_bir_lowering=False, debug=False,
                   num_devices=NCORES)
    x_d = nc.dram_tensor("x", [XROWS + 1, W * B], BF16,
                         kind="ExternalInput").ap()
    w_d = nc.dram_tensor("w", [KP, RPC * WPANEL], BF16,
                         kind="ExternalInput").ap()
    o_d = nc.dram_tensor("o", [B, RPC * OW * CO], BF16,
                         kind="ExternalOutput").ap()

    with tile.TileContext(nc) as tc, ExitStack() as ctx:
        xpool = ctx.enter_context(tc.tile_pool(name="xh", bufs=1))
        wpool = ctx.enter_context(tc.tile_pool(name="wt", bufs=1))
        opool = ctx.enter_context(tc.tile_pool(name="ob", bufs=1))
        ppool = ctx.enter_context(
            tc.tile_pool(name="ps", bufs=8, space=bass.MemorySpace.PSUM))

        xh = [xpool.tile([KP, W * B], BF16, name=f"xh{h}", tag=f"xh{h}")
              for h in range(RPC)]
        wt = wpool.tile([KP, RPC * WPANEL], BF16, name="wt", tag="wt")
        out_sb = opool.tile([B, RPC * OW * CO], BF16, name="ob", tag="ob")

        def issue_inputs(h):
            """Issue row h's input transfers (x main / ones / bias / w)."""
            # single-partition ones/bias rows ride the otherwise-idle
            # gpsimd ring
            nc.gpsimd.dma_start(xh[h][K96:KP, :], x_d[XROWS:XROWS + 1, :])
            nc.gpsimd.dma_start(wt[K96:KP, h * WPANEL:(h + 1) * WPANEL],
                                w_d[K96:KP, h * WPANEL:(h + 1) * WPANEL])
            nc.sync.dma_start(xh[h][0:K96, :], x_d[CI * h:CI * h + K96, :])
            for piece in range(2):
                lo = h * WPANEL + piece * (WPANEL // 2)
                hi = lo + WPANEL // 2
                nc.scalar.dma_start(wt[0:K96, lo:hi], w_d[0:K96, lo:hi])

        # just-in-time issue, one row ahead: row h+1's transfers are issued
        # after row h's first c-group so no matmul's semaphore wait can
        # transitively cover a later row's DMA
        issue_inputs(0)

        for h in range(RPC):
            xt = xh[h]
            wbase = h * WPANEL
            psums = {}
            for c in range(W):
                if c == 2 and h + 1 < RPC:
                    issue_inputs(h + 1)
                lhs = xt[:, c * B:(c + 1) * B]  # [97, 64] stationary
                for seg in segments[c]:
                    i0 = seg[0][0]
                    npair = len(seg)
                    wl0 = seg[0][1][2]
                    beta = wl0 // 8
                    slot0 = wl0 % 8
                    # first write of the whole bank: slot0's j==0 tap (it is
                    # always a single-pair segment since its bank-mates
                    # belong to the previous bank)
                    start = (npair == 1 and seg[0][1][1] == 0 and slot0 == 0)
                    stop = (npair == 1 and seg[0][1][1] == 2
                            and (wl0 % 8 == 7 or wl0 == OW - 1))
                    rhs = wt[:, wbase + i0 * CO:wbase + (i0 + npair) * CO]
                    if beta not in psums:
                        psums[beta] = ppool.tile([B, 512], F32,
                                                 name=f"ps_h{h}_b{beta}",
                                                 tag="ps")
                    out_ap = psums[beta][:, slot0 * CO:(slot0 + npair) * CO]
                    nc.tensor.matmul(out_ap, lhs, rhs, start=start, stop=stop,
                                     skip_group_check=True)
                for beta in DRAIN_AFTER_C.get(c, []):
                    nslot = min(8, OW - 8 * beta)
                    pt = psums.pop(beta)
                    dst = out_sb[:, h * OW * CO + beta * 8 * CO:
                                 h * OW * CO + (beta * 8 + nslot) * CO]
                    # alternate drain engines so neither becomes the pacer
                    if beta % 2 == 0:
                        nc.vector.tensor_copy(dst, pt[:, :nslot * CO])
                    else:
                        nc.scalar.copy(dst, pt[:, :nslot * CO])
                    if beta % 2 == 1:  # banks (beta-1, beta) drained -> ship
                        lo = h * OW * CO + (beta - 1) * 8 * CO
                        hi = h * OW * CO + (beta * 8 + nslot) * CO
                        nc.gpsimd.dma_start(o_d[:, lo:hi], out_sb[:, lo:hi])
    nc.compile()
    return nc


def _prep_inputs(x, weight, bias):
    """Host-side shard + relayout. Returns in_maps for the 8 cores."""
    x = np.ascontiguousarray(np.asarray(x, dtype=np.float32))
    weight = np.ascontiguousarray(np.asarray(weight, dtype=np.float32))
    bias = np.ascontiguousarray(np.asarray(bias, dtype=np.float32))

    x_pad = np.zeros((B, CI, HPAD, W), np.float32)
    x_pad[:, :, :H, :] = x
    # [r, ci, w, b]
    x_t = np.ascontiguousarray(x_pad.transpose(2, 1, 3, 0)).astype(NP_BF16)

    w_pad = np.zeros((OHP, OW, CO, CI, KH, KW), np.float32)
    w_pad[:OH] = weight
    # [oh, kh, ci, wl, kw, o] -> [oh, 96, wl, kw, o]
    w4 = w_pad.transpose(0, 4, 3, 1, 5, 2).reshape(OHP, K96, OW, KW, CO)
    bias_pad = np.zeros((CO, OHP, OW), np.float32)
    bias_pad[:, :OH] = bias
    bias_t = bias_pad.transpose(1, 2, 0)  # [oh, wl, o]

    wl_list = np.array([wl for (c, j, wl) in PAIRS])
    j_list = np.array([j for (c, j, wl) in PAIRS])
    # [oh, 97, 90, 64]: contraction row p = (kh*32 + ci), plus bias row 96
    w2 = np.zeros((OHP, KP, NPAIRS, CO), np.float32)
    w2[:, :K96, :, :] = w4[:, :, wl_list, j_list, :]
    j0 = j_list == 0
    w2[:, K96, j0, :] = bias_t[:, wl_list[j0], :]
    w2 = w2.astype(NP_BF16)

    ones_row = np.ones((1, W * B), NP_BF16)
    in_maps = []
    for core in range(NCORES):
        r0 = RPC * core
        xc = np.ascontiguousarray(np.concatenate(
            [x_t[r0:r0 + RPC + 2].reshape(XROWS, W * B), ones_row], axis=0))
        # [97, (h, pair, co)]
        wc = np.ascontiguousarray(
            w2[r0:r0 + RPC].transpose(1, 0, 2, 3).reshape(KP, RPC * WPANEL))
        in_maps.append({"x": xc, "w": wc})
    return in_maps


def _assemble(results):
    out = np.empty((B, CO, OH, OW), np.float32)
    for core in range(NCORES):
        oc = results[core]["o"].astype(np.float32).reshape(
            B, RPC, OW, CO).transpose(0, 3, 1, 2)
        r0 = RPC * core
        r1 = min(r0 + RPC, OH)
        if r1 > r0:
            out[:, :, r0:r1, :] = oc[:, :, :r1 - r0, :]
    return out


def run(x, weight, bias, trace=False, **trace_kwargs):
    """Build (cached), run on 8 cores, return (output, BassKernelResults)."""
    if "nc" not in _CACHED:
        _CACHED["nc"] = _build_nc()
    nc = _CACHED["nc"]
    in_maps = _prep_inputs(x, weight, bias)
    res = run_bass_kernel_spmd(nc, in_maps, list(range(NCORES)),
                               trace=trace, **trace_kwargs)
    return _assemble(res.results), res


def kernel(x, weight, bias):
    out, _ = run(x, weight, bias)
    return out


# revision 22
# speedup vs baseline: 1.2288x; 1.2288x over previous
"""Trainium2 Bass kernel for a locally-connected Conv2d (nn.Conv2dLocal).

Problem shapes (hardcoded):
  x      [B=64, Cin=32, H=32, W=32]  fp32
  weight [OH=30, OW=30, Cout=64, Cin=32, KH=3, KW=3] fp32 (per-location weights)
  bias   [Cout=64, OH=30, OW=30] fp32
  out    [B=64, Cout=64, OH=30, OW=30] fp32

Strategy: shard the 30 output rows across 8 cores (4 padded rows per core);
all device compute runs in bf16 (inputs quantized on host; rel-err budget is
2e-2, bf16 end-to-end measures ~3.5e-3).

Per core the whole working set fits in SBUF at once, so there is no pool
rotation and every input transfer is dependency-free and issued up front.
Per output row h, an SBUF tile xh[h] holds three input image rows as
[(kh, ci) -> 96 partitions + ones row, (img col c, batch b) free].  For each
image column c the stationary operand xh[:, c] is shared by up to three
(wl, kw) weight taps (wl + kw == c); the per-location weights stream as the
moving operand from one [97, 4*5760] tile (per-h panels side by side),
packed on the host so each step's taps are contiguous.  Accumulation is in
PSUM: one bank holds 8 output locations (64 cols each); per-element
has_written bits make the first tap overwrite and later taps accumulate.
The bias rides as a 97th contraction row against the ones row — matmul time
depends only on the moving free size, so the bias is free on the PE.

DMA rings: sync carries the four [96, 4KB] x mains; scalar carries eight
[96, 5760B] weight half-panels (the PE chases this stream); gpsimd carries
the tiny single-partition ones/bias rows (up front) and the dep-gated per-h
output ships.  96-row transfers fan across all 16 SDMA engines.
"""

import os
import sys

import numpy as np

for _p in ("/opt/trn_rl_repo", "/root/.axon_site/_ro/trn_rl_repo"):
    if os.path.isdir(_p) and _p not in sys.path:
        sys.path.insert(0, _p)

import concourse.bass as bass  # noqa: E402
import concourse.tile as tile  # noqa: E402
from concourse import bacc, mybir  # noqa: E402
from concourse.bass_utils import run_bass_kernel_spmd  # noqa: E402

import ml_dtypes  # noqa: E402

F32 = mybir.dt.float32
BF16 = mybir.dt.bfloat16
NP_BF16 = ml_dtypes.bfloat16

# problem constants
B, CI, H, W = 64, 32, 32, 32
CO = 64
KH = KW = 3
OH = OW = 30
NCORES = 8
RPC = 4  # padded output rows per core (8 * 4 = 32 >= 30)
OHP = NCORES * RPC  # 32
HPAD = OHP + KH - 1  # 34 padded input rows
K96 = KH * CI  # 96 contraction rows per kw tap
KP = K96 + 1  # + ones row for bias
XROWS = (RPC + 2) * CI  # 192 x rows per core (+1 ones row appended)

# (c, j, wl) pair enumeration: j descending within each c so that psum slots
# (wl % 8) ascend within a segment, matching the moving-operand column order.
PAIRS = []
for _c in range(W):
    for _j in (2, 1, 0):
        _wl = _c - _j
        if 0 <= _wl < OW:
            PAIRS.append((_c, _j, _wl))
NPAIRS = len(PAIRS)  # 90
WPANEL = NPAIRS * CO  # 5760 elements per h panel


def _build_segments():
    """Per-c matmul segments: lists of consecutive (pair_idx, (c, j, wl)).

    A segment's taps land in one PSUM bank with ascending slots.  start=True
    is used only for the matmul that is the first write of a whole bank
    (slot 0's j==0 tap) — it clears the bank's has_written bits; hardware's
    per-element has_written bits then make each slot's first tap overwrite
    and later taps accumulate.
    """
    segments = {c: [] for c in range(W)}
    for c in range(W):
        pairs = [(i, PAIRS[i]) for i in range(NPAIRS) if PAIRS[i][0] == c]
        seg = []
        for i, (cc, j, wl) in pairs:
            if seg and (seg[-1][1][2] // 8) != (wl // 8):
                segments[c].append(seg)
                seg = []
            seg.append((i, (cc, j, wl)))
        if seg:
            segments[c].append(seg)
    return segments


# drain bank `beta` right after processing column c == last write for the bank
DRAIN_AFTER_C = {}
for beta in range(4):
    last_wl = min(8 * beta + 7, OW - 1)
    DRAIN_AFTER_C.setdefault(last_wl + 2, []).append(beta)

_CACHED = {}


def _build_nc():
    """Build the single-core SPMD Bass program (identical on all 8 cores)."""
    from contextlib import ExitStack

    segments = _build_segments()
    nc = bacc.Bacc("TRN2", target_bir_lowering=False, debug=False,
                   num_devices=NCORES)
    x_d = nc.dram_tensor("x", [XROWS + 1, W * B], BF16,
                         kind="ExternalInput").ap()
    w_d = nc.dram_tensor("w", [KP, RPC * WPANEL], BF16,
                         kind="ExternalInput").ap()
    o_d = nc.dram_tensor("o", [B, RPC * OW * CO], BF16,
                         kind="ExternalOutput").ap()

    with tile.TileContext(nc) as tc, ExitStack() as ctx:
        xpool = ctx.enter_context(tc.tile_pool(name="xh", bufs=1))
        wpool = ctx.enter_context(tc.tile_pool(name="wt", bufs=1))
        opool = ctx.enter_context(tc.tile_pool(name="ob", bufs=1))
        ppool = ctx.enter_context(
            tc.tile_pool(name="ps", bufs=8, space=bass.MemorySpace.PSUM))

        xh = [xpool.tile([KP, W * B], BF16, name=f"xh{h}", tag=f"xh{h}")
              for h in range(RPC)]
        wt = wpool.tile([KP, RPC * WPANEL], BF16, name="wt", tag="wt")
        out_sb = opool.tile([B, RPC * OW * CO], BF16, name="ob", tag="ob")

        # all input streams are dependency-free and issued up front.
        # gpsimd ring carries ONLY the tiny single-partition ones/bias rows
        # (the dep-gated output ships live on sync so they can never block
        # an input row); scalar streams the weight mains; sync the x mains.
        for h in range(RPC):
            nc.gpsimd.dma_start(xh[h][K96:KP, :], x_d[XROWS:XROWS + 1, :])
            nc.gpsimd.dma_start(wt[K96:KP, h * WPANEL:(h + 1) * WPANEL],
                                w_d[K96:KP, h * WPANEL:(h + 1) * WPANEL])
        for h in range(RPC):
            nc.sync.dma_start(xh[h][0:K96, :], x_d[CI * h:CI * h + K96, :])
        for h in range(RPC):
            # smaller first chunk so the PE starts as early as possible
            npiece = 4 if h == 0 else 2
            for piece in range(npiece):
                lo = h * WPANEL + piece * (WPANEL // npiece)
                hi = lo + WPANEL // npiece
                nc.scalar.dma_start(wt[0:K96, lo:hi], w_d[0:K96, lo:hi])

        for h in range(RPC):
            xt = xh[h]
            wbase = h * WPANEL
            psums = {}
            for c in range(W):
                lhs = xt[:, c * B:(c + 1) * B]  # [97, 64] stationary
                for seg in segments[c]:
                    i0 = seg[0][0]
                    npair = len(seg)
                    wl0 = seg[0][1][2]
                    beta = wl0 // 8
                    slot0 = wl0 % 8
                    # first write of the whole bank: slot0's j==0 tap (it is
                    # always a single-pair segment since its bank-mates
                    # belong to the previous bank)
                    start = (npair == 1 and seg[0][1][1] == 0 and slot0 == 0)
                    stop = (npair == 1 and seg[0][1][1] == 2
                            and (wl0 % 8 == 7 or wl0 == OW - 1))
                    rhs = wt[:, wbase + i0 * CO:wbase + (i0 + npair) * CO]
                    if beta not in psums:
                        psums[beta] = ppool.tile([B, 512], F32,
                                                 name=f"ps_h{h}_b{beta}",
                                                 tag="ps")
                    out_ap = psums[beta][:, slot0 * CO:(slot0 + npair) * CO]
                    nc.tensor.matmul(out_ap, lhs, rhs, start=start, stop=stop,
                                     skip_group_check=True)
                for beta in DRAIN_AFTER_C.get(c, []):
                    nslot = min(8, OW - 8 * beta)
                    pt = psums.pop(beta)
                    dst = out_sb[:, h * OW * CO + beta * 8 * CO:
                                 h * OW * CO + (beta * 8 + nslot) * CO]
                    # alternate drain engines so neither becomes the pacer
                    if beta % 2 == 0:
                        nc.vector.tensor_copy(dst, pt[:, :nslot * CO])
                    else:
                        nc.scalar.copy(dst, pt[:, :nslot * CO])
                    if beta % 2 == 1:  # banks (beta-1, beta) drained -> ship
                        lo = h * OW * CO + (beta - 1) * 8 * CO
                        hi = h * OW * CO + (beta * 8 + nslot) * CO
                        nc.sync.dma_start(o_d[:, lo:hi], out_sb[:, lo:hi])
    nc.compile()
    return nc


def _prep_inputs(x, weight, bias):
    """Host-side shard + relayout. Returns in_maps for the 8 cores."""
    x = np.ascontiguousarray(np.asarray(x, dtype=np.float32))
    weight = np.ascontiguousarray(np.asarray(weight, dtype=np.float32))
    bias = np.ascontiguousarray(np.asarray(bias, dtype=np.float32))

    x_pad = np.zeros((B, CI, HPAD, W), np.float32)
    x_pad[:, :, :H, :] = x
    # [r, ci, w, b]
    x_t = np.ascontiguousarray(x_pad.transpose(2, 1, 3, 0)).astype(NP_BF16)

    w_pad = np.zeros((OHP, OW, CO, CI, KH, KW), np.float32)
    w_pad[:OH] = weight
    # [oh, kh, ci, wl, kw, o] -> [oh, 96, wl, kw, o]
    w4 = w_pad.transpose(0, 4, 3, 1, 5, 2).reshape(OHP, K96, OW, KW, CO)
    bias_pad = np.zeros((CO, OHP, OW), np.float32)
    bias_pad[:, :OH] = bias
    bias_t = bias_pad.transpose(1, 2, 0)  # [oh, wl, o]

    wl_list = np.array([wl for (c, j, wl) in PAIRS])
    j_list = np.array([j for (c, j, wl) in PAIRS])
    # [oh, 97, 90, 64]: contraction row p = (kh*32 + ci), plus bias row 96
    w2 = np.zeros((OHP, KP, NPAIRS, CO), np.float32)
    w2[:, :K96, :, :] = w4[:, :, wl_list, j_list, :]
    j0 = j_list == 0
    w2[:, K96, j0, :] = bias_t[:, wl_list[j0], :]
    w2 = w2.astype(NP_BF16)

    ones_row = np.ones((1, W * B), NP_BF16)
    in_maps = []
    for core in range(NCORES):
        r0 = RPC * core
        xc = np.ascontiguousarray(np.concatenate(
            [x_t[r0:r0 + RPC + 2].reshape(XROWS, W * B), ones_row], axis=0))
        # [97, (h, pair, co)]
        wc = np.ascontiguousarray(
            w2[r0:r0 + RPC].transpose(1, 0, 2, 3).reshape(KP, RPC * WPANEL))
        in_maps.append({"x": xc, "w": wc})
    return in_maps


def _assemble(results):
    out = np.empty((B, CO, OH, OW), np.float32)
    for core in range(NCORES):
        oc = results[core]["o"].astype(np.float32).reshape(
            B, RPC, OW, CO).transpose(0, 3, 1, 2)
        r0 = RPC * core
        r1 = min(r0 + RPC, OH)
        if r1 > r0:
            out[:, :, r0:r1, :] = oc[:, :, :r1 - r0, :]
    return out


def run(x, weight, bias, trace=False, **trace_kwargs):
    """Build (cached), run on 8 cores, return (output, BassKernelResults)."""
    if "nc" not in _CACHED:
        _CACHED["nc"] = _build_nc()
    nc = _CACHED["nc"]
    in_maps = _prep_inputs(x, weight, bias)
    res = run_bass_kernel_spmd(nc, in_maps, list(range(NCORES)),
                               trace=trace, **trace_kwargs)
    return _assemble(res.results), res


def kernel(x, weight, bias):
    out, _ = run(x, weight, bias)
    return out
